# revision 18
# baseline (speedup 1.0000x reference)
"""InternLM3 attention block on 8 Trainium2 NeuronCores (Bass/Tile), v2.

Strategy (tensor-parallel over heads, per the GQA structure):
  - 32 Q heads / 8 KV heads, head_dim 128.  Core c owns Q heads [4c,4c+4)
    and KV head c (one GQA group per core, so K/V never needs replication).
  - All matmul operands bf16 (inputs cast on host): same PE rate as fp32r
    but half the HBM/SBUF traffic and 2x DVE throughput.  PSUM accum f32.
  - Per core, per 512-token block: QKV projection (V produced directly in
    [tok, d] layout via 128-wide sub-matmuls, no transposes) -> RoPE on
    DVE/Pool (bf16, in place) -> causal flash attention in S^T orientation,
    two heads per pass: scores^T for both heads land in one 2-bank PSUM
    tile (double-buffered), one batched exp (ACT) per k-tile, softmax
    denominator accumulated per-head on DVE/Pool, PV accumulated in PSUM
    with the PV matmul one k-step behind the score matmul so PE never
    waits on ACT.
  - Attention outputs are AllGathered across the 8 cores per 512-token
    chunk (bf16); the output projection for chunk i-1 is interleaved into
    block i's attention t-loop to fill PE slack, remainder drains in a
    dense tail.  Each core computes its 512-column slice of out.
"""

import math
import os
import sys

if "/opt/trn_rl_repo" not in sys.path:
    sys.path.insert(0, "/opt/trn_rl_repo")

import ml_dtypes
import numpy as np

import concourse.bass as bass
import concourse.mybir as mybir
import concourse.tile as tile
from concourse import bacc
from concourse import bass_utils

# ---- problem constants (hardcoded per harness contract) ----
HIDDEN = 4096
N_HEADS = 32
N_KV_HEADS = 8
HEAD_DIM = 128
ROPE_THETA = 10000.0
B, S = 2, 2048
NCORES = 8

P = 128
TQ = 512                      # token block
NB = S // TQ                  # 4 blocks per batch
KT = HIDDEN // P              # 32 contraction tiles
KB = 8                        # k-tiles per x DMA batch
QH = N_HEADS // NCORES        # 4 q-heads per core
HG = QH * HEAD_DIM            # 512 = head-group width per core
NCHUNK = B * NB               # 8 allgather chunks
TOK = B * S                   # 4096 tokens
TO = 512                      # outproj token sub-block

f32 = mybir.dt.float32
bf16 = mybir.dt.bfloat16
npbf16 = ml_dtypes.bfloat16


def _build_module(with_collectives=True):
    nc = bacc.Bacc("TRN2", target_bir_lowering=False, debug=False,
                   num_devices=NCORES)
    nc._skip_collectives = not with_collectives

    xT = nc.dram_tensor("xT", [HIDDEN, TOK], bf16, kind="ExternalInput").ap()
    wqkvo = nc.dram_tensor("wqkvo", [HIDDEN, 2 * HG + 2 * HEAD_DIM], bf16,
                           kind="ExternalInput").ap()
    tables = nc.dram_tensor("tables", [P, 2 * S + 4 * TQ + 1], bf16,
                            kind="ExternalInput").ap()
    wqT = wqkvo[:, 0:HG]
    wkT = wqkvo[:, HG:HG + HEAD_DIM]
    wvT = wqkvo[:, HG + HEAD_DIM:HG + 2 * HEAD_DIM]
    woT = wqkvo[:, HG + 2 * HEAD_DIM:]
    cosT = tables[:, 0:S]
    ssinT = tables[:, S:2 * S]
    masksIn = tables[:, 2 * S:2 * S + 4 * TQ]
    onesIn = tables[:, 2 * S + 4 * TQ:]
    outT = nc.dram_tensor("outT", [HG, TOK], f32, kind="ExternalOutput").ap()

    ag_in = [
        nc.dram_tensor(f"ag_in{i}", [HG, TQ], bf16, kind="Internal").ap()
        for i in range(NCHUNK)
    ]
    ag_out = [
        nc.dram_tensor(f"ag_out{i}", [HIDDEN, TQ], bf16, kind="Internal",
                       addr_space="Shared").ap()
        for i in range(NCHUNK)
    ]
    ag7 = {
        "a_in": nc.dram_tensor("ag7a_in", [2 * P, TQ], bf16,
                               kind="Internal").ap(),
        "a_out": nc.dram_tensor("ag7a_out", [16 * P, TQ], bf16,
                                kind="Internal", addr_space="Shared").ap(),
        "b_in": nc.dram_tensor("ag7b_in", [2 * P, TQ], bf16,
                               kind="Internal").ap(),
        "b_out": nc.dram_tensor("ag7b_out", [16 * P, TQ], bf16,
                                kind="Internal", addr_space="Shared").ap(),
    }

    with tile.TileContext(nc) as tc:
        _body(tc, nc, xT, wqT, wkT, wvT, woT, cosT, ssinT, masksIn,
              onesIn, outT, ag_in, ag_out, ag7)
    nc.compile()
    return nc


class OutprojEmitter:
    """Queue of output-projection emission closures, drained op-by-op into
    PE slack inside the attention t-loops (remainder drains densely)."""

    def __init__(self, nc, wo_sb, atpool, obpool, psum, outT, ag_out):
        self.nc = nc
        self.wo_sb = wo_sb
        self.atpool = atpool
        self.obpool = obpool
        self.psum = psum
        self.outT = outT
        self.ag_out = ag_out
        self.q = []
        self.ob_flip = 0

    def add_chunk(self, ch):
        nc = self.nc
        KC = 8  # matmuls per emitted closure
        ats = {}

        def load(half):
            at = self.atpool.tile([P, KT, TO], bf16, tag="at", bufs=1,
                                  name="at")
            nc.sync.dma_start(
                at[:],
                self.ag_out[ch].rearrange("(ko p) t -> p ko t", p=P)[
                    :, :, half * TO:(half + 1) * TO],
            )
            ats[half] = at

        for half in range(TQ // TO):
            self.q.append(lambda half=half: load(half))
        for half in range(TQ // TO):
            for m in range(HG // P):
                def start_chain(half=half, m=m):
                    self._op_ps = self.psum.tile([P, TO], f32, tag="pop",
                                                 bufs=2, name="op_ps")

                def mms(half=half, m=m, k0=0, first=False, last=False):
                    if first:
                        start_chain(half, m)
                    op_ps, at = self._op_ps, ats[half]
                    for k in range(k0, k0 + KC):
                        nc.tensor.matmul(
                            op_ps[:], self.wo_sb[:, k, m * P:(m + 1) * P],
                            at[:, k, :],
                            start=(k == 0), stop=(k == KT - 1),
                        )
                    if last:
                        ob = self.obpool.tile([P, TO], f32, tag="ob", bufs=2,
                                              name="ob")
                        nc.vector.tensor_copy(ob[:], op_ps[:])
                        c0 = ch * TQ + half * TO
                        nc.sync.dma_start(
                            self.outT[m * P:(m + 1) * P, c0:c0 + TO], ob[:])

                for k0 in range(0, KT, KC):
                    self.q.append(
                        lambda half=half, m=m, k0=k0: mms(
                            half, m, k0, first=(k0 == 0),
                            last=(k0 + KC == KT)))

    def add_half7(self, src_out, kmaps, is_b):
        """Outproj for one head-pair half of the final chunk: a 16-ko at
        tile gathered as [core x pair, tok]; kmaps maps local ko -> wo_sb
        ko.  The a-half parks its partial sums in SBUF; the b-half adds
        them to its own and writes outT.  Lets half of the final chunk's
        projection run while attention pass B is still in flight."""
        nc = self.nc
        OP = mybir.AluOpType

        def load():
            at = self.atpool.tile([P, 16, TO], bf16, tag="at", bufs=1,
                                  name="at7")
            nc.sync.dma_start(
                at[:], src_out.rearrange("(ko p) t -> p ko t", p=P))
            self._at7 = at

        self.q.append(load)
        for m in range(HG // P):
            def chain(m=m, is_b=is_b):
                op_ps = self.psum.tile([P, TO], f32, tag="pop", bufs=2,
                                       name="op7")
                at = self._at7
                for j in range(16):
                    nc.tensor.matmul(
                        op_ps[:], self.wo_sb[:, kmaps[j], m * P:(m + 1) * P],
                        at[:, j, :], start=(j == 0), stop=(j == 15))
                if not is_b:
                    nc.vector.tensor_copy(self._ob7a[:, m, :], op_ps[:])
                else:
                    ob = self.obpool.tile([P, TO], f32, tag="ob", bufs=2,
                                          name="ob7")
                    nc.vector.tensor_tensor(
                        ob[:], op_ps[:], self._ob7a[:, m, :], OP.add)
                    c0 = (NCHUNK - 1) * TQ
                    nc.sync.dma_start(
                        self.outT[m * P:(m + 1) * P, c0:c0 + TO], ob[:])
            self.q.append(chain)

    def alloc_ob7a(self):
        self._ob7a = self.obpool.tile([P, HG // P, TO], bf16, tag="ob7a",
                                      bufs=1, name="ob7a")

    def emit(self, n):
        while n > 0 and self.q:
            self.q.pop(0)()
            n -= 1

    def drain(self):
        self.emit(len(self.q))


def _body(tc, nc, xT, wqT, wkT, wvT, woT, cosT, ssinT, masksIn,
          onesIn, outT, ag_in, ag_out, ag7):
    AF = mybir.ActivationFunctionType
    OP = mybir.AluOpType

    with (
        tc.tile_pool(name="wpool", bufs=1) as wpool,
        tc.tile_pool(name="xpool", bufs=2) as xpool,
        tc.tile_pool(name="kvpool", bufs=1) as kvpool,
        tc.tile_pool(name="qpool", bufs=1) as qpool,
        tc.tile_pool(name="epool", bufs=4) as epool,
        tc.tile_pool(name="accpool", bufs=1) as accpool,
        tc.tile_pool(name="aux", bufs=2) as aux,
        tc.tile_pool(name="atpool", bufs=2) as atpool,
        tc.tile_pool(name="obpool", bufs=3) as obpool,
        tc.tile_pool(name="psum", bufs=1, space="PSUM") as psum,
    ):
        # ---- resident constants / weights.  QKV weights + x go on the SP
        # DMA queue interleaved (block 0 starts after ~2MB); everything not
        # needed immediately goes on the ACT DMA queue in parallel.
        wq_sb = wpool.tile([P, KT, HG], bf16, tag="wq")
        wk_sb = wpool.tile([P, KT, HEAD_DIM], bf16, tag="wk")
        wv_sb = wpool.tile([P, KT, HEAD_DIM], bf16, tag="wv")

        def load_wq_chunk(c4):
            nc.sync.dma_start(
                wq_sb[:, c4 * 4:(c4 + 1) * 4, :],
                wqT[c4 * 4 * P:(c4 + 1) * 4 * P, :].rearrange(
                    "(ko p) m -> p ko m", p=P))

        load_wq_chunk(0)
        # block-0 x batches are interleaved with the remaining wq chunks and
        # wk/wv inside the first QKV loop (deferred_w), so PE starts after
        # ~1.5MB of DMA instead of the full weight set.
        deferred_w = [
            lambda: nc.sync.dma_start(
                wk_sb[:], wkT.rearrange("(ko p) m -> p ko m", p=P)),
            lambda: nc.sync.dma_start(
                wv_sb[:], wvT.rearrange("(ko p) m -> p ko m", p=P)),
        ] + [lambda c4=c4: load_wq_chunk(c4) for c4 in range(1, 8)]
        cos_sb = wpool.tile([P, S], bf16, tag="cos")
        nc.scalar.dma_start(cos_sb[:], cosT)
        sin_sb = wpool.tile([P, S], bf16, tag="sin")
        nc.scalar.dma_start(sin_sb[:], ssinT)
        mask_sb = wpool.tile([P, 4, TQ], bf16, tag="mask")
        nc.scalar.dma_start(mask_sb[:], masksIn.rearrange("p (r t) -> p r t", r=4))
        ones_sb = wpool.tile([P, 1], bf16, tag="ones")
        nc.scalar.dma_start(ones_sb[:], onesIn)
        # wo (4MB) is not consumed until the first outproj fillers (~75us
        # in); its dma_start is emitted inside block 0 after the last stage
        # copy so the ACT queue only configures it once block 0's critical
        # x/wq streaming window has drained.
        wo_sb = wpool.tile([P, KT, HG], bf16, tag="wo")

        emitter = OutprojEmitter(nc, wo_sb, atpool, obpool, psum, outT, ag_out)

        def rope(eng, tmptag, dst, n):
            """In-place RoPE on bf16 SBUF tile dst [P, TQ] for block n."""
            cos_blk = cos_sb[:, n * TQ:(n + 1) * TQ]
            sin_blk = sin_sb[:, n * TQ:(n + 1) * TQ]
            rt = aux.tile([P, TQ], bf16, tag=tmptag, bufs=2, name="ropetmp")
            eng.tensor_copy(rt[0:64, :], dst[64:128, :])
            eng.tensor_copy(rt[64:128, :], dst[0:64, :])
            eng.tensor_tensor(rt[:], rt[:], sin_blk, OP.mult)
            eng.tensor_tensor(dst, dst, cos_blk, OP.mult)
            eng.tensor_tensor(dst, dst, rt[:], OP.add)

        def normalize(h, pv_ps, acc, ch, dst=None):
            """softmax denominator + divide for head h, ship to ag_in."""
            dn_ps = psum.tile([1, TQ], f32, tag="pop", bufs=2, name="dn_ps")
            nc.tensor.matmul(dn_ps[:1, :], ones_sb[:], acc[:],
                             start=True, stop=True)
            rec = aux.tile([1, TQ], f32, tag="rec", name="rec")
            nc.vector.reciprocal(rec[:], dn_ps[:1, :])
            bc = aux.tile([P, TQ], f32, tag="bc", name="bc")
            nc.gpsimd.partition_broadcast(bc[:], rec[:])
            ao = aux.tile([P, TQ], bf16, tag="ao", name="ao")
            nc.vector.tensor_tensor(ao[:], pv_ps, bc[:], OP.mult)
            if dst is None:
                dst = ag_in[ch][h * P:(h + 1) * P, :]
            nc.sync.dma_start(dst, ao[:])

        for b in range(B):
            kT_cache = kvpool.tile([P, S], bf16, tag="kT")
            v_cache = kvpool.tile([P, S // P, HEAD_DIM], bf16, tag="v")
            for n in range(NB):
                i_blk = b * NB + n
                ch = i_blk
                tok0 = b * S + n * TQ
                ntk = (n + 1) * (TQ // P)

                # ---------- QKV projection for this token block ----------
                # The whole x block stays resident (two 16-ktile tiles), so
                # the six output chains run sequentially (PSUM accumulation
                # groups are per-bank) and each chain's PSUM drain + RoPE
                # overlaps the later chains.  By the time the k/v chains
                # finish, q0/q1 are already roped, so pass A starts with no
                # boundary stall.
                xa = xpool.tile([P, KT // 2, TQ], bf16, tag="xa", bufs=1, name="xa")
                xb = xpool.tile([P, KT // 2, TQ], bf16, tag="xb", bufs=1, name="xb")
                for xi, xt_ in ((0, xa), (1, xb)):
                    for hh in range(2):
                        ko0 = xi * 16 + hh * 8
                        nc.sync.dma_start(
                            xt_[:, hh * 8:(hh + 1) * 8, :],
                            xT[ko0 * P:(ko0 + 8) * P,
                               tok0:tok0 + TQ].rearrange(
                                "(ko p) t -> p ko t", p=P),
                        )
                        if i_blk == 0:
                            for _ in range(3 if xi == 0 and hh == 0 else 2):
                                if deferred_w:
                                    deferred_w.pop(0)()
                if i_blk >= 1:
                    # at-tile loads for the chunk whose outproj is
                    # interleaved into this block (its AllGather was issued
                    # at the end of the previous block)
                    emitter.add_chunk(i_blk - 1)
                    emitter.emit(1)  # the at-load DMA

                def xk(k):
                    return (xa if k < 16 else xb)[:, k % 16, :]

                qT_sb = qpool.tile([P, QH, TQ], bf16, tag="q", name="qT_sb")
                kblk = kT_cache[:, n * TQ:(n + 1) * TQ]

                q01 = psum.tile([P, 2, TQ], f32, tag="qA", name="q01")
                q23 = psum.tile([P, 2, TQ], f32, tag="qB", name="q23")
                k_ps = psum.tile([P, TQ], f32, tag="kk", name="k_ps")
                if i_blk == 0:
                    # block 0 is DMA-bound (x + weights still streaming in):
                    # run the five q/k chains k-major so PE consumes x at
                    # the pace it arrives instead of sweeping all 32 k-tiles
                    # per chain (the 4 v chains share one PSUM bank so they
                    # stay sequential, after all x has landed).
                    for k in range(KT):
                        st = dict(start=(k == 0), stop=(k == KT - 1))
                        for j in range(2):
                            nc.tensor.matmul(
                                q01[:, j, :], wq_sb[:, k, j * P:(j + 1) * P],
                                xk(k), **st)
                        for j in range(2):
                            nc.tensor.matmul(
                                q23[:, j, :],
                                wq_sb[:, k, (j + 2) * P:(j + 3) * P],
                                xk(k), **st)
                        nc.tensor.matmul(k_ps[:], wk_sb[:, k, :], xk(k), **st)
                    nc.scalar.copy(qT_sb[:, 0:2, :], q01[:])
                    rope(nc.vector, "rtD", qT_sb[:, 0, :], n)
                    rope(nc.vector, "rtD", qT_sb[:, 1, :], n)
                    nc.scalar.copy(qT_sb[:, 2:4, :], q23[:])
                    rope(nc.vector, "rtD", qT_sb[:, 2, :], n)
                    rope(nc.vector, "rtD", qT_sb[:, 3, :], n)
                    nc.scalar.copy(kblk, k_ps[:])
                    rope(nc.vector, "rtD", kblk, n)
                else:
                    for j in range(2):
                        for k in range(KT):
                            nc.tensor.matmul(
                                q01[:, j, :], wq_sb[:, k, j * P:(j + 1) * P],
                                xk(k), start=(k == 0), stop=(k == KT - 1))
                    nc.scalar.copy(qT_sb[:, 0:2, :], q01[:])
                    rope(nc.vector, "rtD", qT_sb[:, 0, :], n)
                    rope(nc.vector, "rtD", qT_sb[:, 1, :], n)

                    for j in range(2):
                        for k in range(KT):
                            nc.tensor.matmul(
                                q23[:, j, :],
                                wq_sb[:, k, (j + 2) * P:(j + 3) * P],
                                xk(k), start=(k == 0), stop=(k == KT - 1))
                    nc.scalar.copy(qT_sb[:, 2:4, :], q23[:])
                    rope(nc.vector, "rtD", qT_sb[:, 2, :], n)
                    rope(nc.vector, "rtD", qT_sb[:, 3, :], n)

                    for k in range(KT):
                        nc.tensor.matmul(k_ps[:], wk_sb[:, k, :], xk(k),
                                         start=(k == 0), stop=(k == KT - 1))
                    nc.scalar.copy(kblk, k_ps[:])
                    rope(nc.vector, "rtD", kblk, n)

                v4 = psum.tile([P, 4, HEAD_DIM], f32, tag="vv", name="v4")
                for j in range(4):
                    for k in range(KT):
                        nc.tensor.matmul(
                            v4[:, j, :], xk(k)[:, j * P:(j + 1) * P],
                            wv_sb[:, k, :], start=(k == 0), stop=(k == KT - 1))
                nc.scalar.copy(
                    v_cache[:, n * 4:(n + 1) * 4, :], v4[:])
                if i_blk == 0:
                    nc.scalar.dma_start(
                        wo_sb[:], woT.rearrange("(ko p) m -> p ko m", p=P))

                # ---------- attention: two heads per pass ----------
                for pas in range(2):
                    h0, h1 = 2 * pas, 2 * pas + 1
                    pv0 = psum.tile([P, TQ], f32, tag="kk", name="pv0")
                    pv1 = psum.tile([P, TQ], f32, tag="vv", name="pv1")
                    acc0 = accpool.tile([P, TQ], bf16, tag=f"acc{h0}",
                                        name="acc0")
                    acc1 = accpool.tile([P, TQ], bf16, tag=f"acc{h1}",
                                        name="acc1")
                    es_prev = None
                    po = 0
                    for t in range(ntk):
                        emitter.emit(1)
                        # diagonal k-tile r: q columns < r*128 are fully
                        # masked, so compute only the valid sub-range
                        # (kept full for the 4-tile n=0 blocks where the
                        # range would touch the accumulation start).
                        r = t - (ntk - 4)
                        qo = r * P if (r > 0 and ntk > 4) else 0
                        st2 = psum.tile([P, 2, TQ], f32,
                                        tag=("qA" if t % 2 == 0 else "qB"),
                                        name="st2")
                        kt = kT_cache[:, t * P:(t + 1) * P]
                        nc.tensor.matmul(st2[:, 0, qo:], kt,
                                         qT_sb[:, h0, qo:],
                                         start=True, stop=True)
                        nc.tensor.matmul(st2[:, 1, qo:], kt,
                                         qT_sb[:, h1, qo:],
                                         start=True, stop=True)
                        if es_prev is not None:
                            tp = t - 1
                            nc.tensor.matmul(
                                pv0[:, po:], v_cache[:, tp, :],
                                es_prev[:, 0, po:],
                                start=(tp == 0), stop=False)
                            nc.tensor.matmul(
                                pv1[:, po:], v_cache[:, tp, :],
                                es_prev[:, 1, po:],
                                start=(tp == 0), stop=False)
                        es2 = epool.tile([P, 2, TQ], bf16, tag="es",
                                         name="es2")
                        nc.scalar.activation(es2[:, :, qo:], st2[:, :, qo:],
                                             AF.Exp)
                        if r >= 0:
                            # with the sub-range in play only the first 128
                            # valid columns need masking; the full-range
                            # n=0 blocks still need the full-width mask
                            mw = TQ - qo if qo == 0 and r > 0 else P
                            mk = mask_sb[:, r, qo:qo + mw]
                            nc.vector.tensor_tensor(
                                es2[:, 0, qo:qo + mw], es2[:, 0, qo:qo + mw],
                                mk, OP.mult)
                            nc.vector.tensor_tensor(
                                es2[:, 1, qo:qo + mw], es2[:, 1, qo:qo + mw],
                                mk, OP.mult)
                        if t == 0:
                            nc.vector.tensor_copy(acc0[:], es2[:, 0, :])
                            nc.gpsimd.tensor_copy(acc1[:], es2[:, 1, :])
                        else:
                            nc.vector.tensor_tensor(
                                acc0[:, qo:], acc0[:, qo:], es2[:, 0, qo:],
                                OP.add)
                            nc.gpsimd.tensor_tensor(
                                acc1[:, qo:], acc1[:, qo:], es2[:, 1, qo:],
                                OP.add)
                        es_prev = es2
                        po = qo
                    tp = ntk - 1
                    nc.tensor.matmul(pv0[:, po:], v_cache[:, tp, :],
                                     es_prev[:, 0, po:],
                                     start=(tp == 0), stop=True)
                    nc.tensor.matmul(pv1[:, po:], v_cache[:, tp, :],
                                     es_prev[:, 1, po:],
                                     start=(tp == 0), stop=True)
                    last = ch == NCHUNK - 1
                    if last:
                        buf = ag7["a_in"] if pas == 0 else ag7["b_in"]
                        normalize(h0, pv0[:], acc0, ch,
                                  dst=buf[(h0 % 2) * P:(h0 % 2) * P + P, :])
                        normalize(h1, pv1[:], acc1, ch,
                                  dst=buf[(h1 % 2) * P:(h1 % 2) * P + P, :])
                    else:
                        normalize(h0, pv0[:], acc0, ch)
                        normalize(h1, pv1[:], acc1, ch)
                    if last and pas == 0:
                        if not getattr(nc, "_skip_collectives", False):
                            nc.gpsimd.collective_compute(
                                "AllGather", mybir.AluOpType.bypass,
                                replica_groups=[list(range(NCORES))],
                                ins=[ag7["a_in"].opt()],
                                outs=[ag7["a_out"].opt()])
                        emitter.alloc_ob7a()
                        emitter.add_half7(
                            ag7["a_out"],
                            [(j // 2) * 4 + (j % 2) for j in range(16)],
                            is_b=False)

                # ---------- AllGather this chunk across the 8 cores ----
                if not getattr(nc, "_skip_collectives", False):
                    if ch < NCHUNK - 1:
                        nc.gpsimd.collective_compute(
                            "AllGather",
                            mybir.AluOpType.bypass,
                            replica_groups=[list(range(NCORES))],
                            ins=[ag_in[ch].opt()],
                            outs=[ag_out[ch].opt()],
                        )
                    else:
                        nc.gpsimd.collective_compute(
                            "AllGather", mybir.AluOpType.bypass,
                            replica_groups=[list(range(NCORES))],
                            ins=[ag7["b_in"].opt()],
                            outs=[ag7["b_out"].opt()])

        # ---------- tail: remaining output projection ----------
        emitter.add_half7(
            ag7["b_out"],
            [(j // 2) * 4 + 2 + (j % 2) for j in range(16)], is_b=True)
        emitter.drain()


_NC_CACHE = None


def _get_module():
    global _NC_CACHE
    if _NC_CACHE is None:
        _NC_CACHE = _build_module(
            with_collectives=not bool(int(os.environ.get("KERNEL_NO_CC", "0"))))
    return _NC_CACHE


def _host_consts():
    inv_freq = 1.0 / (ROPE_THETA ** (np.arange(0, HEAD_DIM, 2,
                                               dtype=np.float32) / HEAD_DIM))
    t = np.arange(S, dtype=np.float32)
    freqs = np.outer(t, inv_freq).astype(np.float32)      # [S, 64]
    cos_h = np.cos(freqs).T                               # [64, S]
    sin_h = np.sin(freqs).T
    cosT = np.concatenate([cos_h, cos_h], axis=0)
    ssinT = np.concatenate([-sin_h, sin_h], axis=0)

    i = np.arange(P)[:, None]
    j = np.arange(TQ)[None, :]
    masks = np.concatenate(
        [(i + r * P <= j).astype(np.float32) for r in range(4)], axis=1
    )                                                     # [128, 4*512]
    ones = np.ones((P, 1), dtype=np.float32)
    return (cosT.astype(npbf16), ssinT.astype(npbf16),
            masks.astype(npbf16), ones.astype(npbf16))


def make_in_maps(hidden_states, wq, wk, wv, wo):
    hidden_states = np.asarray(hidden_states, dtype=np.float32)
    wq = np.asarray(wq, dtype=np.float32)
    wk = np.asarray(wk, dtype=np.float32)
    wv = np.asarray(wv, dtype=np.float32)
    wo = np.asarray(wo, dtype=np.float32)

    xT = np.ascontiguousarray(
        hidden_states.reshape(TOK, HIDDEN).T).astype(npbf16)
    cosT, ssinT, masks, ones = _host_consts()
    qscale = 1.0 / math.sqrt(HEAD_DIM)
    tables = np.concatenate(
        [cosT, ssinT, masks, ones], axis=1).astype(npbf16)

    in_maps = []
    for c in range(NCORES):
        wqkvo = np.concatenate([
            (wq[c * HG:(c + 1) * HG] * qscale).T,
            wk[c * HEAD_DIM:(c + 1) * HEAD_DIM].T,
            wv[c * HEAD_DIM:(c + 1) * HEAD_DIM].T,
            wo[c * HG:(c + 1) * HG].T,
        ], axis=1).astype(npbf16)
        in_maps.append({
            "xT": xT,
            "wqkvo": np.ascontiguousarray(wqkvo),
            "tables": tables,
        })
    return in_maps


def assemble_output(results):
    out = np.empty((TOK, HIDDEN), dtype=np.float32)
    for c in range(NCORES):
        out[:, c * HG:(c + 1) * HG] = results[c]["outT"].T
    return out.reshape(B, S, HIDDEN)


def kernel(hidden_states, wq, wk, wv, wo):
    nc = _get_module()
    in_maps = make_in_maps(hidden_states, wq, wk, wv, wo)
    trace = bool(int(os.environ.get("KERNEL_TRACE", "0")))
    res = bass_utils.run_bass_kernel_spmd(
        nc, in_maps, core_ids=list(range(NCORES)), trace=trace
    )
    if trace:
        kernel.last_results = res
    return assemble_output(res.results)


kernel.last_results = None
